# revision 5
# baseline (speedup 1.0000x reference)
# DKVMN Trainium2 Bass kernel (v4: tensor_tensor_scan recurrence).
#
# Sharding: data-parallel over batch across 8 NeuronCores (8 sequences each);
# embedding tables and all parameters replicated.
#
# Per-core program (bs = t*8 + b, "t-major", BS=1600):
#   P1  q2c_table/q2c_mask rows gathered by question id with ap_gather on all
#       8 gpsimd cores (16-partition channel blocks, 200 indices each), then
#       reassembled to [4, BS, 2] via a DRAM bounce.
#   P2  index math on DVE; masked entries redirect to out-of-range ids.
#   P3  indices/correctness/1-den broadcast to all 128 partitions via PE
#       rank-1 matmuls (ones x row) + ACT copies.
#   P4  one-hot COUNT matrices by iota-compare on DVE (fp16 4x mode) over
#       4 chunks of 125 concept rows; value-table counts derived from the
#       key counts by correctness masking.  Embedding gathers become PE
#       matmuls over the natural-layout tables.
#   P6  w = softmax(kbar^T Mk^T), batched; single batched reciprocal.
#   P7  e/a = sigmoid/tanh(vbar^T W^T + b); ACT scatters the results into
#       b-major tiles (col = b*200 + t) so the P8 builds see t contiguous.
#   P8  recurrence via tensor_tensor_scan: per (slot n, seq b) group the
#       free dim holds [reset, t0..t19]; A' = 1 - w*e (gpsimd mult + ACT
#       scale=-1/bias=1), B' = w*a (DVE) with B'[g,0] = chunk carry.  One
#       scan instruction per 20-step chunk advances all 400 groups; reads
#       are one batched multiply vs the shifted trajectory plus an add-tree
#       over slots, all in DVE 2x mode.
#   P9  f = tanh([reads, kbar] f_W^T + f_b); out = sigmoid(f p_W^T + p_b).
import sys

for _p in ("/opt/trn_rl_repo", "/root/.axon_site/_ro/trn_rl_repo"):
    if _p not in sys.path:
        sys.path.append(_p)

from contextlib import ExitStack

import numpy as np
import ml_dtypes

import concourse.bass as bass
import concourse.bacc as bacc
import concourse.mybir as mybir
from concourse.bass_utils import run_bass_kernel_spmd
from concourse.tile import TileContext

F32 = mybir.dt.float32
BF16 = mybir.dt.bfloat16
FP16 = mybir.dt.float16
I32 = mybir.dt.int32
I16 = mybir.dt.int16
AF = mybir.ActivationFunctionType
OP = mybir.AluOpType

B, S, DK, SLOTS = 64, 200, 128, 50
NUM_Q, NUM_C, MAXC = 10000, 500, 4
NCORES = 8
BL = B // NCORES          # 8 sequences per core
BS = BL * S               # 1600 (bs = t*BL + b)
NB = SLOTS * BL           # 400 recurrence groups (g = n*8 + b)
CP = 125                  # concept rows per table chunk (500 = 4*125)
KCH = 4                   # key table chunks
VCH = 8                   # value table chunks (1000 = 8*125)
NCH = (BS + 127) // 128   # 13 bs-chunks for softmax
TCH = 20                  # recurrence chunk length (steps)
NCHK = S // TCH           # 10 chunks
GW = TCH + 1              # group width in the scan layout (reset + 20)
NIX = 208                 # padded per-gpsimd-core gather index count

_PROG = None  # cached compiled program


def _build_program():
    nc = bacc.Bacc("TRN2", target_bir_lowering=False, debug=False,
                   num_devices=NCORES)

    def din(name, shape, dt):
        return nc.dram_tensor(name, shape, dt, kind="ExternalInput")

    qseq_w = din("qseq_w", [128, NIX // 16], I16)
    corrf = din("corrf", [4, BS], F32)
    q2c_comb = din("q2c_comb", [4, 2 * NUM_Q], I16)
    ket_d = din("ket", [128, KCH * DK], FP16)
    vet_d = din("vet", [128, VCH * DK], FP16)
    iof_d = din("iof", [128, 1], F32)
    mkt_d = din("mkt", [DK, SLOTS], FP16)
    ewt_d = din("ewt", [DK, DK], FP16)
    awt_d = din("awt", [DK, DK], FP16)
    fw1t_d = din("fw1t", [DK, DK], FP16)
    fw2t_d = din("fw2t", [DK, DK], FP16)
    pwt_d = din("pwt", [DK, 1], FP16)
    eb_d = din("eb", [DK, 1], F32)
    ab_d = din("ab", [DK, 1], F32)
    fb_d = din("fb", [DK, 1], F32)
    pb_d = din("pb", [1, 1], F32)
    mv0_d = din("mv0r", [DK, NB], FP16)
    out_d = nc.dram_tensor("out", [1, BS], F32, kind="ExternalOutput")

    with ExitStack() as ctx:
        ctx.enter_context(
            nc.allow_low_precision("bf16 state; rel-err budget 2e-2"))
        tc = ctx.enter_context(TileContext(nc))
        const = ctx.enter_context(tc.tile_pool(name="const", bufs=1))
        main = ctx.enter_context(tc.tile_pool(name="main", bufs=1))
        dram = ctx.enter_context(tc.tile_pool(name="dram", bufs=1,
                                              space="DRAM"))

        # ---- persistent tiles ----
        kbar = main.tile([DK, BS], FP16, tag="kbar")
        e_b = main.tile([DK, BS], FP16, tag="e_b")      # b-major (b*200+t)
        a_b = main.tile([DK, BS], FP16, tag="a_b")      # b-major
        idb = main.tile([DK, BS], FP16, tag="idb")
        w_rows = main.tile([128, NCH, SLOTS], FP16, tag="w_rows")
        reads_bs = main.tile([DK, BS], FP16, tag="reads_bs")  # t-major
        f_all = main.tile([DK, BS], FP16, tag="f_all")
        out_sb = main.tile([1, BS], F32, tag="out_sb")

        # ---- params (const pool) ----
        kes = const.tile([128, KCH, DK], FP16, tag="kes")
        ves = const.tile([128, VCH, DK], FP16, tag="ves")
        iof = const.tile([128, 1], F32, tag="iof")
        mkt = const.tile([DK, SLOTS], FP16, tag="mkt")
        ewt = const.tile([DK, DK], FP16, tag="ewt")
        awt = const.tile([DK, DK], FP16, tag="awt")
        fw1t = const.tile([DK, DK], FP16, tag="fw1t")
        fw2t = const.tile([DK, DK], FP16, tag="fw2t")
        pwt = const.tile([DK, 1], FP16, tag="pwt")
        eb = const.tile([DK, 1], F32, tag="eb")
        ab = const.tile([DK, 1], F32, tag="ab")
        fb = const.tile([DK, 1], F32, tag="fb")
        pb = const.tile([1, 1], F32, tag="pb")
        selcs = const.tile([4, 4, 4], F32, tag="selcs")
        quarter = const.tile([4, DK], F32, tag="quarter")
        onesel = const.tile([4, 4, DK], FP16, tag="onesel")
        ones128 = const.tile([128, DK], FP16, tag="ones128")
        nc.vector.memset(quarter[...], 0.25)
        nc.vector.memset(selcs[...], 0.0)
        for j in range(4):
            nc.vector.tensor_scalar(onesel[:, j, :],
                                    iof[0:4, :].broadcast_to([4, DK]),
                                    float(j), None, op0=OP.is_equal)
            nc.vector.memset(selcs[:, j, j:j + 1], 1.0)
        nc.vector.memset(ones128[...], 1.0)
        nc.sync.dma_start(kes[...],
                          ket_d[...].rearrange("p (c d) -> p c d", c=KCH))
        nc.sync.dma_start(ves[...],
                          vet_d[...].rearrange("p (c d) -> p c d", c=VCH))
        for tile_, dt_ in ((iof, iof_d), (mkt, mkt_d), (ewt, ewt_d),
                           (awt, awt_d), (fw1t, fw1t_d), (fw2t, fw2t_d),
                           (pwt, pwt_d), (eb, eb_d), (ab, ab_d), (fb, fb_d),
                           (pb, pb_d)):
            nc.sync.dma_start(tile_[...], dt_[...])

        # gpsimd gather-library warm-up: a dummy 16-index gather forces the
        # Q7 microcode load to overlap the input DMAs.
        dg_t = const.tile([16, 2, 2], I16, tag="dg_t")
        dg_i = const.tile([16, 1], I16, tag="dg_i")
        dg_o = const.tile([16, 1, 2], I16, tag="dg_o")
        nc.vector.memset(dg_t[...], 0)
        nc.vector.memset(dg_i[...], 0)
        nc.gpsimd.ap_gather(dg_o[...], dg_t[...], dg_i[...], channels=16,
                            num_elems=2, d=2, num_idxs=16)

        psA_stack = ExitStack()
        psA = psA_stack.enter_context(
            tc.tile_pool(name="psA", bufs=1, space="PSUM"))

        pfB_stack = ExitStack()
        pfB = pfB_stack.enter_context(tc.tile_pool(name="pfB", bufs=1))
        kbi = pfB.tile([128, KCH, BS], FP16, tag="kbi")
        corrh = pfB.tile([128, BS], FP16, tag="corrh")
        cnt = pfB.tile([128, KCH, BS], FP16, tag="cnt")
        wvm = pfB.tile([128, VCH, BS], FP16, tag="wvm")
        nc.vector.memset(cnt[...], 0.0)
        nc.vector.memset(wvm[...], 0.0)
        isq = pfB.tile([CP, 4, BS], FP16, tag="isq")
        s01 = pfB.tile([CP, 2, BS], FP16, tag="s01")
        iotc = pfB.tile([CP, 1], F32, tag="iotc")
        vbar = pfB.tile([DK, BS], FP16, tag="vbar")

        with tc.tile_pool(name="pfA", bufs=1) as pfA:
            # ---- P1: gather cids/mask rows on all 8 gpsimd cores ----
            # channel block k (partitions 16k..16k+15) handles bs slice
            # [200k, 200k+200); rows 16k+j (j<4) hold table column j.
            q2c_t = pfA.tile([128, NUM_Q, 2], I16, tag="q2c")
            qw = pfA.tile([128, NIX // 16], I16, tag="qw")
            for k in range(8):
                nc.sync.dma_start(q2c_t[16 * k:16 * k + 4, :, :],
                                  q2c_comb[...].rearrange(
                                      "p (q e) -> p q e", e=2))
            nc.sync.dma_start(qw[...], qseq_w[...])
            qc_g = pfA.tile([128, NIX, 2], I16, tag="qc_g")
            nc.gpsimd.ap_gather(qc_g[...], q2c_t[...], qw[...], channels=128,
                                num_elems=NUM_Q, d=2, num_idxs=NIX)
            # warm the gpsimd vector-op library for the P8 A-builds
            dg_a = pfA.tile([1, 4], FP16, tag="dg_a")
            dg_b = pfA.tile([1, 4], FP16, tag="dg_b")
            nc.vector.memset(dg_a[...], 1.0)
            nc.gpsimd.tensor_mul(dg_b[...], dg_a[...], dg_a[...])

            # reassemble to qc[4, BS, 2] via a DRAM bounce
            qtmp = dram.tile([128 * NIX * 2], I16, tag="qtmp")
            nc.sync.dma_start(
                qtmp[...].rearrange("(p x) -> p x", p=128),
                qc_g[...].rearrange("p i e -> p (i e)"))
            qc = pfA.tile([4, BS, 2], I16, tag="qc")
            nc.sync.dma_start(
                qc[...].rearrange("p (k i) e -> p k i e", k=8),
                qtmp[...].rearrange("(k p i e) -> p k i e",
                                    k=8, p=16, e=2)[0:4, :, 0:200, :])

            # ---- P2: index math (f32, exact) ----
            corr = pfA.tile([4, BS], F32, tag="corr")
            nc.sync.dma_start(corr[...], corrf[...])
            cidsf = pfA.tile([4, BS], F32, tag="cidsf")
            mskf = pfA.tile([4, BS], F32, tag="mskf")
            nc.vector.tensor_copy(cidsf[...], qc[0:4, :, 0])
            nc.vector.tensor_copy(mskf[...], qc[0:4, :, 1])
            # masked entries -> id 500: outside every 125-row chunk, so
            # they contribute no counts
            k1 = pfA.tile([4, BS], F32, tag="k1")
            nc.vector.scalar_tensor_tensor(k1[...], cidsf[...], -500.0,
                                           mskf[...], op0=OP.add, op1=OP.mult)
            kh = pfA.tile([4, BS], FP16, tag="kh")
            nc.vector.tensor_scalar_add(kh[...], k1[...], 500.0)

            # den = max(sum_j mask, 1); one batched reciprocal on [4, 400]
            # (chunk c's column sums land on partition c via selcs[:, c, :])
            inv4 = pfA.tile([4, 400], FP16, tag="inv4")
            msum_ps = psA.tile([4, 400], F32, tag="mm1s")
            for c in range(4):
                sl = slice(c * 400, (c + 1) * 400)
                nc.tensor.matmul(msum_ps[...], selcs[:, c, :], mskf[:, sl],
                                 start=(c == 0), stop=(c == 3))
            den4 = pfA.tile([4, 400], F32, tag="den4")
            nc.vector.tensor_scalar_max(den4[...], msum_ps[...], 1.0)
            nc.vector.reciprocal(inv4[...], den4[...])

            # ---- P3: broadcasts via PE rank-1 matmuls + ACT copies ----
            for s in range(4):
                sl = slice(s * 400, (s + 1) * 400)
                for j in range(4):
                    kp = psA.tile([128, 400], F32, tag="mm2", bufs=4)
                    nc.tensor.matmul(kp[...], onesel[:, j, :],
                                     kh[:, sl])
                    nc.scalar.activation(kbi[:, j, sl], kp[...], AF.Copy)
                cp_ = psA.tile([128, 400], F32, tag="mm2", bufs=4)
                nc.tensor.matmul(cp_[...], quarter[...], corr[:, sl])
                nc.scalar.activation(corrh[:, sl], cp_[...], AF.Copy)
                ip_ = psA.tile([128, 400], F32, tag="mm2", bufs=4)
                nc.tensor.matmul(ip_[...], onesel[:, s, :], inv4[...])
                nc.scalar.activation(idb[:, sl], ip_[...], AF.Copy)

        # ---- P4: count matrices by iota-compare; PE "gathers" ----
        kbi3 = kbi[0:CP, :, :]
        for c in range(KCH):
            nc.vector.tensor_scalar_add(iotc[...], iof[0:CP, :],
                                        float(CP * c))
            nc.vector.tensor_scalar(isq[...], kbi3, iotc[...], None,
                                    op0=OP.is_equal)
            nc.vector.tensor_add(s01[...], isq[:, 0:2, :], isq[:, 2:4, :])
            nc.vector.tensor_add(cnt[0:CP, c, :], s01[:, 0, :], s01[:, 1, :])
        for c in range(KCH):
            # value counts: chunk c gets correct=0 mass, chunk 4+c correct=1
            nc.vector.tensor_mul(wvm[0:CP, KCH + c, :], cnt[0:CP, c, :],
                                 corrh[0:CP, :])
            nc.vector.tensor_sub(wvm[0:CP, c, :], cnt[0:CP, c, :],
                                 wvm[0:CP, KCH + c, :])

        for s in range(4):
            sl = slice(s * 400, (s + 1) * 400)
            kb_ps = psA.tile([DK, 400], F32, tag="mm2", bufs=4)
            for c in range(KCH):
                nc.tensor.matmul(kb_ps[...], kes[:, c, :], cnt[:, c, sl],
                                 start=(c == 0), stop=(c == KCH - 1))
            nc.vector.tensor_mul(kbar[:, sl], kb_ps[...], idb[:, sl])
            vb_ps = psA.tile([DK, 400], F32, tag="mm2", bufs=4)
            for c in range(VCH):
                nc.tensor.matmul(vb_ps[...], ves[:, c, :], wvm[:, c, sl],
                                 start=(c == 0), stop=(c == VCH - 1))
            nc.vector.tensor_mul(vbar[:, sl], vb_ps[...], idb[:, sl])

        # ---- P6: w = softmax(kbar^T @ Mk^T), batched ----
        # 64-slot padding keeps every matmul output inside one PSUM bank
        lg = psA.tile([128, NCH, 64], F32, tag="mm3", bufs=1)
        for c in range(NCH):
            p = min(128, BS - c * 128)
            nc.tensor.matmul(lg[:p, c, 0:SLOTS],
                             kbar[:, c * 128:c * 128 + p], mkt[...])
        ex = pfB.tile([128, NCH, SLOTS], F32, tag="ex")
        nc.scalar.activation(ex[...], lg[:, :, 0:SLOTS], AF.Exp)
        t25 = pfB.tile([128, NCH, 25], F32, tag="t25")
        t12 = pfB.tile([128, NCH, 12], F32, tag="t12")
        t6 = pfB.tile([128, NCH, 6], F32, tag="t6")
        t3 = pfB.tile([128, NCH, 3], F32, tag="t3")
        sx = pfB.tile([128, NCH, 1], F32, tag="sx")
        rx = pfB.tile([128, NCH], F32, tag="rx")
        nc.vector.tensor_add(t25[...], ex[:, :, 0:25], ex[:, :, 25:50])
        nc.vector.tensor_add(t12[...], t25[:, :, 0:12], t25[:, :, 12:24])
        nc.vector.tensor_add(t6[...], t12[:, :, 0:6], t12[:, :, 6:12])
        nc.vector.tensor_add(t3[...], t6[:, :, 0:3], t6[:, :, 3:6])
        nc.vector.tensor_add(sx[...], t3[:, :, 0:1], t3[:, :, 1:2])
        nc.vector.tensor_add(sx[...], sx[...], t3[:, :, 2:3])
        nc.vector.tensor_add(sx[...], sx[...], t25[:, :, 24:25])
        nc.vector.reciprocal(rx[...], sx[:, :, 0])
        nc.vector.tensor_tensor(
            w_rows[...], ex[...],
            rx[...].unsqueeze(2).broadcast_to([128, NCH, SLOTS]), OP.mult)

        # reorder w into per-step rows via DRAM bounce (loaded per chunk)
        wdram = dram.tile([NCH * 128 * SLOTS], FP16, tag="wdram")
        nc.sync.dma_start(
            wdram[...].rearrange("(c p n) -> p c n", p=128, n=SLOTS),
            w_rows[...])

        # ---- P7: e/a (ACT scatters into b-major tiles) ----
        for c in range(4):
            sl = slice(c * 400, (c + 1) * 400)
            # dst view: cols bs = t*8+b of this block land at b*200 + t
            ebv = e_b[...].rearrange("p (b t) -> p t b", b=BL)[
                :, c * 50:(c + 1) * 50, :]
            abv = a_b[...].rearrange("p (b t) -> p t b", b=BL)[
                :, c * 50:(c + 1) * 50, :]
            ep = psA.tile([DK, 400], F32, tag="mm2", bufs=4)
            nc.tensor.matmul(ep[...], ewt[...], vbar[:, sl])
            nc.scalar.activation(
                ebv, ep[...].rearrange("p (t b) -> p t b", b=BL),
                AF.Sigmoid, bias=eb[...], scale=1.0)
            ap_ = psA.tile([DK, 400], F32, tag="mm2", bufs=4)
            nc.tensor.matmul(ap_[...], awt[...], vbar[:, sl])
            nc.scalar.activation(
                abv, ap_[...].rearrange("p (t b) -> p t b", b=BL),
                AF.Tanh, bias=ab[...], scale=1.0)

        pfB_stack.close()
        psA_stack.close()

        # ---- P8: recurrence via tensor_tensor_scan ----
        # Scan layout: free dim = 400 groups (g = n*8+b) x GW (reset + 20
        # steps).  state = A'[f]*state + B'[f]; A'[g,0]=0 so B'[g,0] (the
        # chunk carry) reloads the state at each group boundary.  The scan
        # output Y[g, 0:20] is exactly Mv_{t-1} for the 20 reads.
        with ExitStack() as rstk:
            pr = rstk.enter_context(tc.tile_pool(name="pr", bufs=1))
            psW = rstk.enter_context(
                tc.tile_pool(name="psW", bufs=1, space="PSUM"))

            w32c = [pr.tile([128, 8, NB], FP16, tag=f"w32c{i}",
                            name=f"w32c{i}") for i in range(2)]
            wkb = [pr.tile([128, TCH * NB], FP16, tag=f"wkb{i}",
                           name=f"wkb{i}") for i in range(2)]
            Ab = [pr.tile([128, GW * NB], FP16, tag=f"Ab{i}",
                          name=f"Ab{i}") for i in range(2)]
            Bb = [pr.tile([128, GW * NB], FP16, tag=f"Bb{i}",
                          name=f"Bb{i}") for i in range(2)]
            Yb = pr.tile([128, GW * NB], FP16, tag="Yb")
            p0b = pr.tile([128, TCH * NB], FP16, tag="p0b")
            tr1 = pr.tile([128, 25 * BL * TCH], FP16, tag="tr1")
            tr2 = pr.tile([128, 12 * BL * TCH], FP16, tag="tr2")
            tr3 = pr.tile([128, 6 * BL * TCH], FP16, tag="tr3")
            tr4 = pr.tile([128, 3 * BL * TCH], FP16, tag="tr4")
            tr5 = pr.tile([128, BL * TCH], FP16, tag="tr5")
            tr6 = pr.tile([128, BL * TCH], FP16, tag="tr6")

            def gview(t):  # [128, 50, 8, GW] group view
                return t[...].rearrange("p (n b s) -> p n b s",
                                        n=SLOTS, b=BL)

            def wview(t):  # [128, 50, 8, TCH]
                return t[...].rearrange("p (n b j) -> p n b j",
                                        n=SLOTS, b=BL)

            # A' reset slots stay 0 forever (builds write slots 1..20 only)
            for i in range(2):
                nc.vector.memset(
                    Ab[i][...].rearrange("p (g s) -> p g s", s=GW)[:, :, 0],
                    0.0)
            # chunk-0 carry = Mv0
            nc.sync.dma_start(
                Bb[0][...].rearrange("p (g s) -> p g s", s=GW)[:, :, 0],
                mv0_d[...])

            eb3 = e_b[...].rearrange("p (b t) -> p b t", b=BL)
            ab3 = a_b[...].rearrange("p (b t) -> p b t", b=BL)

            def emit_wchunk(k):
                # load per-step w rows and broadcast to all 128 partitions,
                # scattering into the group layout (j innermost).
                wk, wc = wkb[k % 2], w32c[k % 2]
                u0 = (k * TCH) // 3
                for k3 in range(3):
                    base = (3 * u0 + k3) * NB
                    span = min(8 * 3 * NB, NCH * 128 * SLOTS - base)
                    nu = span // (3 * NB)
                    src = wdram[base:base + nu * 3 * NB] \
                        .rearrange("(u j bn) -> u j bn", j=3, bn=NB)[:, 0, :]
                    nc.sync.dma_start(wc[32 * k3:32 * k3 + 1, 0:nu, :], src)
                for g in range(TCH // 4):
                    wbps = psW.tile([128, 4 * 512], F32, tag="wbps", bufs=2,
                                    name=f"wbps{k}_{g}")
                    for s4 in range(4):
                        t = k * TCH + g * 4 + s4
                        al = 32 * (t % 3)
                        nc.tensor.matmul(
                            wbps[:, 512 * s4:512 * s4 + NB],
                            ones128[al:al + 1, :],
                            wc[al:al + 1, t // 3 - u0, :])
                    src = wbps[...].rearrange("p (s x) -> p s x", s=4)[
                        :, :, 0:NB].rearrange("p s (n b) -> p s n b", n=SLOTS)
                    dst = wview(wk)[:, :, :, g * 4:g * 4 + 4] \
                        .rearrange("p n b s -> p s n b")
                    nc.scalar.activation(dst, src, AF.Copy)

            def emit_builds(k):
                # A' = 1 - w*e: gpsimd mult (one chunk of slack), ACT fixes
                # in-place with scale=-1 bias=1.  B' = w*a on DVE.
                wk = wkb[k % 2]
                Av = gview(Ab[k % 2])[:, :, :, 1:GW]
                Bv = gview(Bb[k % 2])[:, :, :, 1:GW]
                ebc = eb3[:, :, k * TCH:(k + 1) * TCH].unsqueeze(1) \
                    .broadcast_to([128, SLOTS, BL, TCH])
                abc = ab3[:, :, k * TCH:(k + 1) * TCH].unsqueeze(1) \
                    .broadcast_to([128, SLOTS, BL, TCH])
                nc.gpsimd.tensor_mul(Av, wview(wk), ebc)
                nc.scalar.activation(Av, Av, AF.Copy, bias=1.0, scale=-1.0)
                nc.vector.tensor_tensor(Bv, wview(wk), abc, OP.mult)

            emit_wchunk(0)
            emit_builds(0)
            for k in range(NCHK):
                nc.vector.tensor_tensor_scan(
                    Yb[...], Ab[k % 2][...], Bb[k % 2][...], 0.0,
                    OP.mult, OP.add)
                if k + 1 < NCHK:
                    emit_wchunk(k + 1)
                    # carry: B'_{k+1}[g, 0] = Y_k[g, 20]
                    nc.vector.tensor_copy(
                        Bb[(k + 1) % 2][...]
                        .rearrange("p (g s) -> p g s", s=GW)[:, :, 0],
                        Yb[...].rearrange("p (g s) -> p g s",
                                          s=GW)[:, :, TCH])
                # reads: p0 = w_t * Mv_{t-1} (shifted trajectory), then
                # add-tree over n
                p0v = wview(p0b)
                yv = gview(Yb)[:, :, :, 0:TCH]
                nc.vector.tensor_tensor(p0v, wview(wkb[k % 2]), yv, OP.mult)
                t1v = tr1[...].rearrange("p (n b j) -> p n b j", n=25, b=BL)
                t2v = tr2[...].rearrange("p (n b j) -> p n b j", n=12, b=BL)
                t3v = tr3[...].rearrange("p (n b j) -> p n b j", n=6, b=BL)
                t4v = tr4[...].rearrange("p (n b j) -> p n b j", n=3, b=BL)
                t5v = tr5[...].rearrange("p (b j) -> p b j", b=BL)
                t6v = tr6[...].rearrange("p (b j) -> p b j", b=BL)
                nc.vector.tensor_add(t1v, p0v[:, 0:25], p0v[:, 25:50])
                nc.vector.tensor_add(t2v, t1v[:, 0:12], t1v[:, 12:24])
                nc.vector.tensor_add(t3v, t2v[:, 0:6], t2v[:, 6:12])
                nc.vector.tensor_add(t4v, t3v[:, 0:3], t3v[:, 3:6])
                nc.vector.tensor_add(t5v, t4v[:, 0], t4v[:, 1])
                nc.vector.tensor_add(t6v, t5v, t4v[:, 2])
                rdv = reads_bs[...].rearrange("p (t b) -> p b t", b=BL)[
                    :, :, k * TCH:(k + 1) * TCH]
                nc.vector.tensor_add(rdv, t6v, t1v[:, 24])
                if k + 1 < NCHK:
                    emit_builds(k + 1)

        # ---- P9: output head ----
        psB_stack = ExitStack()
        psB = psB_stack.enter_context(
            tc.tile_pool(name="psB", bufs=1, space="PSUM"))
        for c in range(4):
            sl = slice(c * 400, (c + 1) * 400)
            fp = psB.tile([DK, 400], F32, tag="mm2", bufs=4)
            nc.tensor.matmul(fp[...], fw1t[...], reads_bs[:, sl],
                             start=True, stop=False)
            nc.tensor.matmul(fp[...], fw2t[...], kbar[:, sl],
                             start=False, stop=True)
            nc.scalar.activation(f_all[:, sl], fp[...], AF.Tanh,
                                 bias=fb[...], scale=1.0)
        for c in range(4):
            sl = slice(c * 400, (c + 1) * 400)
            pp = psB.tile([1, 400], F32, tag="mm1", bufs=2)
            nc.tensor.matmul(pp[...], pwt[...], f_all[:, sl])
            nc.scalar.activation(out_sb[:, sl], pp[...], AF.Sigmoid,
                                 bias=pb[...], scale=1.0)
        nc.sync.dma_start(out_d[...], out_sb[...])
        psB_stack.close()

    nc.finalize()
    return nc


def _host_inputs(inputs):
    """Build per-core + replicated DRAM inputs from the full problem inputs."""
    bf = np.float16
    qs = np.asarray(inputs["question_seq"]).astype(np.int64)
    cs = np.asarray(inputs["correctness_seq"]).astype(np.int64)
    q2c = np.asarray(inputs["q2c_table"]).astype(np.int32)
    q2m = np.asarray(inputs["q2c_mask"]).astype(np.int32)
    ke = np.asarray(inputs["key_embed"], np.float32)
    ve = np.asarray(inputs["value_embed"], np.float32)
    mk = np.asarray(inputs["Mk"], np.float32)
    mv0 = np.asarray(inputs["Mv0"], np.float32)
    fw = np.asarray(inputs["f_W"], np.float32)
    fb = np.asarray(inputs["f_b"], np.float32)
    ew = np.asarray(inputs["e_W"], np.float32)
    eb = np.asarray(inputs["e_b"], np.float32)
    aw = np.asarray(inputs["a_W"], np.float32)
    ab = np.asarray(inputs["a_b"], np.float32)
    pw = np.asarray(inputs["p_W"], np.float32)
    pb = np.asarray(inputs["p_b"], np.float32)

    # [CP, C*DK] chunked-contiguous table layouts (chunk c rows 125c..125c+124)
    kep = np.zeros((128, KCH, DK), np.float16)
    kep[0:CP] = ke.astype(np.float16).reshape(KCH, CP, DK).transpose(1, 0, 2)
    kep = kep.reshape(128, KCH * DK)
    vep = np.zeros((128, VCH, DK), np.float16)
    vep[0:CP] = ve.astype(np.float16).reshape(VCH, CP, DK).transpose(1, 0, 2)
    vep = vep.reshape(128, VCH * DK)

    rep = {
        "q2c_comb": np.stack([q2c.T, q2m.T], 2).reshape(4, 2 * NUM_Q)
        .astype(np.int16),
        "ket": np.ascontiguousarray(kep),
        "vet": np.ascontiguousarray(vep),
        "iof": np.arange(128, dtype=np.float32).reshape(128, 1),
        "mkt": mk.T.astype(bf),
        "ewt": ew.T.astype(bf),
        "awt": aw.T.astype(bf),
        "fw1t": fw[:, :DK].T.astype(bf),
        "fw2t": fw[:, DK:].T.astype(bf),
        "pwt": pw.T.astype(bf),
        "eb": eb.reshape(DK, 1).astype(np.float32),
        "ab": ab.reshape(DK, 1).astype(np.float32),
        "fb": fb.reshape(DK, 1).astype(np.float32),
        "pb": pb.reshape(1, 1).astype(np.float32),
        "mv0r": np.repeat(mv0.T, BL, axis=1).astype(bf),
    }
    in_maps = []
    for core in range(NCORES):
        q_flat = qs[core * BL:(core + 1) * BL].T.reshape(-1)   # t-major
        c_flat = cs[core * BL:(core + 1) * BL].T.reshape(-1)
        # per-gpsimd-core index lists: core k takes bs [200k, 200k+200),
        # padded to NIX and wrapped into its 16 partitions
        qwa = np.zeros((8, NIX), np.int16)
        qwa[:, 0:200] = q_flat.reshape(8, 200)
        qwa = qwa.reshape(8, NIX // 16, 16).transpose(0, 2, 1) \
            .reshape(128, NIX // 16)
        m = dict(rep)
        m["qseq_w"] = np.ascontiguousarray(qwa)
        m["corrf"] = np.broadcast_to(c_flat.astype(np.float32),
                                     (4, BS)).copy()
        in_maps.append(m)
    return in_maps


def kernel(**inputs):
    global _PROG
    if _PROG is None:
        _PROG = _build_program()
    in_maps = _host_inputs(inputs)
    res = run_bass_kernel_spmd(_PROG, in_maps, core_ids=list(range(NCORES)))
    out = np.zeros((B, S), np.float32)
    for core in range(NCORES):
        o = res.results[core]["out"].reshape(S, BL)
        out[core * BL:(core + 1) * BL] = o.T
    return out


# revision 8
# speedup vs baseline: 1.7206x; 1.7206x over previous
# DKVMN Trainium2 Bass kernel (v5).
#
# Sharding: data-parallel over batch across 8 NeuronCores (8 sequences each);
# embedding tables and all parameters replicated.
#
# Per-core program (bs = t*8 + b, "t-major", BS=1600):
#   P1  q2c_table/q2c_mask rows gathered by question id with ap_gather on all
#       8 gpsimd cores (16-partition channel blocks, 200 indices each), then
#       reassembled to [4, BS, 2] via a DRAM bounce.  The gather microcode
#       library is pre-warmed by a dummy gather so its ~45us Q7 load overlaps
#       the input DMAs.
#   P2  index math on DVE; masked entries redirect to out-of-range ids
#       (500/1000) that no count chunk matches.  One batched reciprocal on
#       [4, 400] replaces four serial [1, 400] reciprocals.
#   P3  indices/correctness/1-den broadcast to all 128 partitions via PE
#       rank-1 matmuls (ones x row) + ACT copies.
#   P4  one-hot COUNT matrices by iota-compare on DVE (fp16, 4x mode) over
#       4 chunks of 125 concept rows; value-table counts derived from the
#       key counts by correctness masking (500 = 4*125 keeps chunks aligned).
#       Embedding gathers become PE matmuls over the natural-layout tables.
#   P6  w = softmax(kbar^T Mk^T), batched: one PE pass into PSUM, one exp,
#       tree-sum over slots, one reciprocal, one scaled multiply.
#   P7  e/a = sigmoid/tanh(vbar^T W^T + b) (PE + ACT), t-major contiguous.
#   P8  recurrence Mv_t = Mv_{t-1} * (1 - w e^T) + w a^T over 10 chunks of
#       20 steps, everything t-outer so every DVE op runs in 2x mode:
#       PE broadcasts w rows into PSUM, ACT copies them contiguously to
#       SBUF, DVE builds A = 1 - w*e directly in A2 (TT mult + in-place ACT
#       bias=1) and the FULL B = w*a (no gpsimd B => no chain stall), then
#       a 2-TT-per-step chain advances the state in-place; p0 = w * Mv_{t-1}
#       stays on DVE (2x) while the whole add-tree over slots runs on the
#       otherwise-idle gpsimd engine, off the critical path.
#   P9  f = tanh([reads, kbar] f_W^T + f_b); out = sigmoid(f p_W^T + p_b).
import sys

for _p in ("/opt/trn_rl_repo", "/root/.axon_site/_ro/trn_rl_repo"):
    if _p not in sys.path:
        sys.path.append(_p)

from contextlib import ExitStack

import numpy as np
import ml_dtypes

import concourse.bass as bass
import concourse.bacc as bacc
import concourse.mybir as mybir
from concourse.bass_utils import run_bass_kernel_spmd
from concourse.tile import TileContext

F32 = mybir.dt.float32
BF16 = mybir.dt.bfloat16
FP16 = mybir.dt.float16
I32 = mybir.dt.int32
I16 = mybir.dt.int16
AF = mybir.ActivationFunctionType
OP = mybir.AluOpType

B, S, DK, SLOTS = 64, 200, 128, 50
NUM_Q, NUM_C, MAXC = 10000, 500, 4
NCORES = 8
BL = B // NCORES          # 8 sequences per core
BS = BL * S               # 1600 (bs = t*BL + b)
NB = SLOTS * BL           # 400 state columns per step (n-major, b-inner)
CP = 125                  # concept rows per table chunk (500 = 4*125)
KCH = 4                   # key table chunks
VCH = 8                   # value table chunks (1000 = 8*125)
NCH = (BS + 127) // 128   # 13 bs-chunks for softmax
TCH = 20                  # recurrence chunk length (steps)
NCHK = S // TCH           # 10 chunks
NIX = 208                 # padded per-gpsimd-core gather index count

_PROG = None  # cached compiled program


def _build_program():
    nc = bacc.Bacc("TRN2", target_bir_lowering=False, debug=False,
                   num_devices=NCORES)

    def din(name, shape, dt):
        return nc.dram_tensor(name, shape, dt, kind="ExternalInput")

    qseq_w = din("qseq_w", [128, NIX // 16], I16)
    corrf = din("corrf", [4, BS], F32)
    q2c_comb = din("q2c_comb", [4, 2 * NUM_Q], I16)
    ket_d = din("ket", [128, KCH * DK], FP16)
    vet_d = din("vet", [128, VCH * DK], FP16)
    iof_d = din("iof", [128, 1], F32)
    mkt_d = din("mkt", [DK, SLOTS], FP16)
    ewt_d = din("ewt", [DK, DK], FP16)
    awt_d = din("awt", [DK, DK], FP16)
    fw1t_d = din("fw1t", [DK, DK], FP16)
    fw2t_d = din("fw2t", [DK, DK], FP16)
    pwt_d = din("pwt", [DK, 1], FP16)
    eb_d = din("eb", [DK, 1], F32)
    ab_d = din("ab", [DK, 1], F32)
    fb_d = din("fb", [DK, 1], F32)
    pb_d = din("pb", [1, 1], F32)
    mv0_d = din("mv0r", [DK, NB], FP16)
    out_d = nc.dram_tensor("out", [1, BS], F32, kind="ExternalOutput")

    with ExitStack() as ctx:
        ctx.enter_context(
            nc.allow_low_precision("bf16 state; rel-err budget 2e-2"))
        tc = ctx.enter_context(TileContext(nc))
        const = ctx.enter_context(tc.tile_pool(name="const", bufs=1))
        main = ctx.enter_context(tc.tile_pool(name="main", bufs=1))
        dram = ctx.enter_context(tc.tile_pool(name="dram", bufs=1,
                                              space="DRAM"))

        # ---- persistent tiles ----
        kbar = main.tile([DK, BS], FP16, tag="kbar")
        e_all = main.tile([DK, BS], FP16, tag="e_all")
        a_all = main.tile([DK, BS], FP16, tag="a_all")
        idb = main.tile([DK, BS], FP16, tag="idb")
        w_rows = main.tile([128, NCH, SLOTS], FP16, tag="w_rows")
        reads_bs = main.tile([DK, BS], FP16, tag="reads_bs")
        f_all = main.tile([DK, BS], FP16, tag="f_all")
        out_sb = main.tile([1, BS], F32, tag="out_sb")

        # ---- params (const pool) ----
        kes = const.tile([128, KCH, DK], FP16, tag="kes")
        ves = const.tile([128, VCH, DK], FP16, tag="ves")
        iof = const.tile([128, 1], F32, tag="iof")
        mkt = const.tile([DK, SLOTS], FP16, tag="mkt")
        ewt = const.tile([DK, DK], FP16, tag="ewt")
        awt = const.tile([DK, DK], FP16, tag="awt")
        fw1t = const.tile([DK, DK], FP16, tag="fw1t")
        fw2t = const.tile([DK, DK], FP16, tag="fw2t")
        pwt = const.tile([DK, 1], FP16, tag="pwt")
        eb = const.tile([DK, 1], F32, tag="eb")
        ab = const.tile([DK, 1], F32, tag="ab")
        fb = const.tile([DK, 1], F32, tag="fb")
        pb = const.tile([1, 1], F32, tag="pb")
        selcs = const.tile([4, 4, 4], F32, tag="selcs")
        quarter = const.tile([4, DK], F32, tag="quarter")
        onesel = const.tile([4, 4, DK], FP16, tag="onesel")
        ones128 = const.tile([128, DK], FP16, tag="ones128")
        nc.vector.memset(quarter[...], 0.25)
        nc.vector.memset(selcs[...], 0.0)
        for j in range(4):
            nc.vector.tensor_scalar(onesel[:, j, :],
                                    iof[0:4, :].broadcast_to([4, DK]),
                                    float(j), None, op0=OP.is_equal)
            nc.vector.memset(selcs[:, j, j:j + 1], 1.0)
        nc.vector.memset(ones128[...], 1.0)
        nc.sync.dma_start(kes[...],
                          ket_d[...].rearrange("p (c d) -> p c d", c=KCH))
        nc.sync.dma_start(ves[...],
                          vet_d[...].rearrange("p (c d) -> p c d", c=VCH))
        for tile_, dt_ in ((iof, iof_d), (mkt, mkt_d), (ewt, ewt_d),
                           (awt, awt_d), (fw1t, fw1t_d), (fw2t, fw2t_d),
                           (pwt, pwt_d), (eb, eb_d), (ab, ab_d), (fb, fb_d),
                           (pb, pb_d)):
            nc.sync.dma_start(tile_[...], dt_[...])

        # gpsimd gather-library warm-up: a dummy 16-index gather forces the
        # Q7 microcode load to overlap the input DMAs.
        dg_t = const.tile([16, 2, 2], I16, tag="dg_t")
        dg_i = const.tile([16, 1], I16, tag="dg_i")
        dg_o = const.tile([16, 1, 2], I16, tag="dg_o")
        nc.vector.memset(dg_t[...], 0)
        nc.vector.memset(dg_i[...], 0)
        nc.gpsimd.ap_gather(dg_o[...], dg_t[...], dg_i[...], channels=16,
                            num_elems=2, d=2, num_idxs=16)

        psA_stack = ExitStack()
        psA = psA_stack.enter_context(
            tc.tile_pool(name="psA", bufs=1, space="PSUM"))

        pfB_stack = ExitStack()
        pfB = pfB_stack.enter_context(tc.tile_pool(name="pfB", bufs=1))
        kbi = pfB.tile([128, KCH, BS], FP16, tag="kbi")
        corrh = pfB.tile([128, BS], FP16, tag="corrh")
        cnt = pfB.tile([128, KCH, BS], FP16, tag="cnt")
        wvm = pfB.tile([128, VCH, BS], FP16, tag="wvm")
        nc.vector.memset(cnt[...], 0.0)
        nc.vector.memset(wvm[...], 0.0)
        isq = pfB.tile([CP, 4, BS], FP16, tag="isq")
        s01 = pfB.tile([CP, 2, BS], FP16, tag="s01")
        iotc = pfB.tile([CP, 1], F32, tag="iotc")
        vbar = pfB.tile([DK, BS], FP16, tag="vbar")

        with tc.tile_pool(name="pfA", bufs=1) as pfA:
            # ---- P1: gather cids/mask rows on all 8 gpsimd cores ----
            # channel block k (partitions 16k..16k+15) handles bs slice
            # [200k, 200k+200); rows 16k+j (j<4) hold table column j.
            q2c_t = pfA.tile([128, NUM_Q, 2], I16, tag="q2c")
            qw = pfA.tile([128, NIX // 16], I16, tag="qw")
            for k in range(8):
                nc.sync.dma_start(q2c_t[16 * k:16 * k + 4, :, :],
                                  q2c_comb[...].rearrange(
                                      "p (q e) -> p q e", e=2))
            nc.sync.dma_start(qw[...], qseq_w[...])
            qc_g = pfA.tile([128, NIX, 2], I16, tag="qc_g")
            nc.gpsimd.ap_gather(qc_g[...], q2c_t[...], qw[...], channels=128,
                                num_elems=NUM_Q, d=2, num_idxs=NIX)
            # warm the gpsimd vector-op library for the P8 tree adds
            dg_a = pfA.tile([1, 4], FP16, tag="dg_a")
            dg_b = pfA.tile([1, 4], FP16, tag="dg_b")
            nc.vector.memset(dg_a[...], 1.0)
            nc.gpsimd.tensor_mul(dg_b[...], dg_a[...], dg_a[...])

            # reassemble to qc[4, BS, 2] via a DRAM bounce
            qtmp = dram.tile([128 * NIX * 2], I16, tag="qtmp")
            nc.sync.dma_start(
                qtmp[...].rearrange("(p x) -> p x", p=128),
                qc_g[...].rearrange("p i e -> p (i e)"))
            qc = pfA.tile([4, BS, 2], I16, tag="qc")
            nc.sync.dma_start(
                qc[...].rearrange("p (k i) e -> p k i e", k=8),
                qtmp[...].rearrange("(k p i e) -> p k i e",
                                    k=8, p=16, e=2)[0:4, :, 0:200, :])

            # ---- P2: index math (f32, exact) ----
            corr = pfA.tile([4, BS], F32, tag="corr")
            nc.sync.dma_start(corr[...], corrf[...])
            cidsf = pfA.tile([4, BS], F32, tag="cidsf")
            mskf = pfA.tile([4, BS], F32, tag="mskf")
            nc.vector.tensor_copy(cidsf[...], qc[0:4, :, 0])
            nc.vector.tensor_copy(mskf[...], qc[0:4, :, 1])
            # masked entries -> id 500: outside every 125-row chunk, so
            # they contribute no counts
            k1 = pfA.tile([4, BS], F32, tag="k1")
            nc.vector.scalar_tensor_tensor(k1[...], cidsf[...], -500.0,
                                           mskf[...], op0=OP.add, op1=OP.mult)
            kh = pfA.tile([4, BS], FP16, tag="kh")
            nc.vector.tensor_scalar_add(kh[...], k1[...], 500.0)

            # den = max(sum_j mask, 1); one batched reciprocal on [4, 400]
            # (chunk c's column sums land on partition c via selcs[:, c, :])
            inv4 = pfA.tile([4, 400], FP16, tag="inv4")
            msum_ps = psA.tile([4, 400], F32, tag="mm1s")
            for c in range(4):
                sl = slice(c * 400, (c + 1) * 400)
                nc.tensor.matmul(msum_ps[...], selcs[:, c, :], mskf[:, sl],
                                 start=(c == 0), stop=(c == 3))
            den4 = pfA.tile([4, 400], F32, tag="den4")
            nc.vector.tensor_scalar_max(den4[...], msum_ps[...], 1.0)
            nc.vector.reciprocal(inv4[...], den4[...])

            # ---- P3: broadcasts via PE rank-1 matmuls + ACT copies ----
            for s in range(4):
                sl = slice(s * 400, (s + 1) * 400)
                for j in range(4):
                    kp = psA.tile([128, 400], F32, tag="mm2", bufs=4)
                    nc.tensor.matmul(kp[...], onesel[:, j, :],
                                     kh[:, sl])
                    nc.scalar.activation(kbi[:, j, sl], kp[...], AF.Copy)
                cp_ = psA.tile([128, 400], F32, tag="mm2", bufs=4)
                nc.tensor.matmul(cp_[...], quarter[...], corr[:, sl])
                nc.scalar.activation(corrh[:, sl], cp_[...], AF.Copy)
                ip_ = psA.tile([128, 400], F32, tag="mm2", bufs=4)
                nc.tensor.matmul(ip_[...], onesel[:, s, :], inv4[...])
                nc.scalar.activation(idb[:, sl], ip_[...], AF.Copy)

        # ---- P4: count matrices by iota-compare; PE "gathers" ----
        kbi3 = kbi[0:CP, :, :]
        for c in range(KCH):
            nc.vector.tensor_scalar_add(iotc[...], iof[0:CP, :],
                                        float(CP * c))
            nc.vector.tensor_scalar(isq[...], kbi3, iotc[...], None,
                                    op0=OP.is_equal)
            nc.vector.tensor_add(s01[...], isq[:, 0:2, :], isq[:, 2:4, :])
            nc.vector.tensor_add(cnt[0:CP, c, :], s01[:, 0, :], s01[:, 1, :])
        for c in range(KCH):
            # value counts: chunk c gets correct=0 mass, chunk 4+c correct=1
            nc.vector.tensor_mul(wvm[0:CP, KCH + c, :], cnt[0:CP, c, :],
                                 corrh[0:CP, :])
            nc.vector.tensor_sub(wvm[0:CP, c, :], cnt[0:CP, c, :],
                                 wvm[0:CP, KCH + c, :])

        for s in range(4):
            sl = slice(s * 400, (s + 1) * 400)
            kb_ps = psA.tile([DK, 400], F32, tag="mm2", bufs=4)
            for c in range(KCH):
                nc.tensor.matmul(kb_ps[...], kes[:, c, :], cnt[:, c, sl],
                                 start=(c == 0), stop=(c == KCH - 1))
            nc.vector.tensor_mul(kbar[:, sl], kb_ps[...], idb[:, sl])
            vb_ps = psA.tile([DK, 400], F32, tag="mm2", bufs=4)
            for c in range(VCH):
                nc.tensor.matmul(vb_ps[...], ves[:, c, :], wvm[:, c, sl],
                                 start=(c == 0), stop=(c == VCH - 1))
            nc.vector.tensor_mul(vbar[:, sl], vb_ps[...], idb[:, sl])

        # ---- P6: w = softmax(kbar^T @ Mk^T), batched ----
        # 64-slot padding keeps every matmul output inside one PSUM bank
        lg = psA.tile([128, NCH, 64], F32, tag="mm3", bufs=1)
        for c in range(NCH):
            p = min(128, BS - c * 128)
            nc.tensor.matmul(lg[:p, c, 0:SLOTS],
                             kbar[:, c * 128:c * 128 + p], mkt[...])
        ex = pfB.tile([128, NCH, SLOTS], F32, tag="ex")
        nc.scalar.activation(ex[...], lg[:, :, 0:SLOTS], AF.Exp)
        t25 = pfB.tile([128, NCH, 25], F32, tag="t25")
        t12 = pfB.tile([128, NCH, 12], F32, tag="t12")
        t6 = pfB.tile([128, NCH, 6], F32, tag="t6")
        t3 = pfB.tile([128, NCH, 3], F32, tag="t3")
        sx = pfB.tile([128, NCH, 1], F32, tag="sx")
        rx = pfB.tile([128, NCH], F32, tag="rx")
        nc.vector.tensor_add(t25[...], ex[:, :, 0:25], ex[:, :, 25:50])
        nc.vector.tensor_add(t12[...], t25[:, :, 0:12], t25[:, :, 12:24])
        nc.vector.tensor_add(t6[...], t12[:, :, 0:6], t12[:, :, 6:12])
        nc.vector.tensor_add(t3[...], t6[:, :, 0:3], t6[:, :, 3:6])
        nc.vector.tensor_add(sx[...], t3[:, :, 0:1], t3[:, :, 1:2])
        nc.vector.tensor_add(sx[...], sx[...], t3[:, :, 2:3])
        nc.vector.tensor_add(sx[...], sx[...], t25[:, :, 24:25])
        nc.vector.reciprocal(rx[...], sx[:, :, 0])
        nc.vector.tensor_tensor(
            w_rows[...], ex[...],
            rx[...].unsqueeze(2).broadcast_to([128, NCH, SLOTS]), OP.mult)

        # reorder w into per-step rows via DRAM bounce (loaded per chunk)
        wdram = dram.tile([NCH * 128 * SLOTS], FP16, tag="wdram")
        nc.sync.dma_start(
            wdram[...].rearrange("(c p n) -> p c n", p=128, n=SLOTS),
            w_rows[...])

        # ---- P7: e/a (t-major contiguous) ----
        for c in range(4):
            sl = slice(c * 400, (c + 1) * 400)
            ep = psA.tile([DK, 400], F32, tag="mm2", bufs=4)
            nc.tensor.matmul(ep[...], ewt[...], vbar[:, sl])
            nc.scalar.activation(e_all[:, sl], ep[...], AF.Sigmoid,
                                 bias=eb[...], scale=1.0)
            ap_ = psA.tile([DK, 400], F32, tag="mm2", bufs=4)
            nc.tensor.matmul(ap_[...], awt[...], vbar[:, sl])
            nc.scalar.activation(a_all[:, sl], ap_[...], AF.Tanh,
                                 bias=ab[...], scale=1.0)
        # ne = -e so A = 1 - w*e becomes a single TT mult + in-place ACT +1
        ne_all = main.tile([DK, BS], FP16, tag="ne_all")
        nc.vector.tensor_scalar_mul(ne_all[...], e_all[...], -1.0)

        pfB_stack.close()
        psA_stack.close()

        # ---- P8: recurrence, t-outer chunked chain ----
        # Everything 2x-mode on DVE.  A = w*(-e) (TT) then +1 in-place on
        # ACT; B = w*a fully on DVE (no gpsimd producer in the chain's
        # dependency cone).  The per-step chain is split into two
        # independent half-width column streams so consecutive DVE
        # instructions alternate streams and the dependent-op semaphore
        # latency is hidden.  p0 = w*Mv_{t-1} runs on DVE right after the
        # chain; the add-tree over slots runs on gpsimd during the next
        # chunk, completely off the DVE critical path.
        HB = NB // 2
        with ExitStack() as rstk:
            pr = rstk.enter_context(tc.tile_pool(name="pr", bufs=1))
            psW = rstk.enter_context(
                tc.tile_pool(name="psW", bufs=1, space="PSUM"))

            w32c = [pr.tile([128, 8, NB], FP16, tag=f"w32c{i}",
                            name=f"w32c{i}") for i in range(2)]
            wsb = [pr.tile([128, TCH * NB], FP16, tag=f"wsb{i}",
                           name=f"wsb{i}") for i in range(2)]
            A2 = [pr.tile([128, TCH * NB], FP16, tag=f"A2{i}",
                          name=f"A2{i}") for i in range(2)]
            B2 = [pr.tile([128, TCH * NB], FP16, tag=f"B2{i}",
                          name=f"B2{i}") for i in range(2)]
            st = pr.tile([128, (TCH + 1) * NB], FP16, tag="st")
            p0t = pr.tile([128, TCH * NB], FP16, tag="p0t")
            m2 = [pr.tile([128, HB], FP16, tag=f"m2{i}", name=f"m2{i}")
                  for i in range(4)]
            tr1 = pr.tile([128, TCH * 25 * BL], FP16, tag="tr1")
            tr2 = pr.tile([128, TCH * 12 * BL], FP16, tag="tr2")
            tr3 = pr.tile([128, TCH * 6 * BL], FP16, tag="tr3")
            tr4 = pr.tile([128, TCH * 3 * BL], FP16, tag="tr4")
            tr5 = pr.tile([128, TCH * BL], FP16, tag="tr5")
            tr6 = pr.tile([128, TCH * BL], FP16, tag="tr6")
            mv0s = pr.tile([DK, NB], FP16, tag="mv0s")
            nc.sync.dma_start(mv0s[...], mv0_d[...])
            nc.vector.tensor_copy(st[:, 0:NB], mv0s[...])

            e3 = ne_all[...].rearrange("p (t b) -> p t b", b=BL)
            a3 = a_all[...].rearrange("p (t b) -> p t b", b=BL)

            def emit_wchunk(k):
                wk, wc = wsb[k % 2], w32c[k % 2]
                u0 = (k * TCH) // 3
                for k3 in range(3):
                    base = (3 * u0 + k3) * NB
                    span = min(8 * 3 * NB, NCH * 128 * SLOTS - base)
                    nu = span // (3 * NB)
                    src = wdram[base:base + nu * 3 * NB] \
                        .rearrange("(u j bn) -> u j bn", j=3, bn=NB)[:, 0, :]
                    nc.sync.dma_start(wc[32 * k3:32 * k3 + 1, 0:nu, :], src)
                for g in range(TCH // 4):
                    wbps = psW.tile([128, 4 * 512], F32, tag="wbps", bufs=2,
                                    name=f"wbps{k}_{g}")
                    for s4 in range(4):
                        t = k * TCH + g * 4 + s4
                        al = 32 * (t % 3)
                        nc.tensor.matmul(
                            wbps[:, 512 * s4:512 * s4 + NB],
                            ones128[al:al + 1, :],
                            wc[al:al + 1, t // 3 - u0, :])
                    nc.scalar.activation(
                        wk[:, g * 4 * NB:(g + 1) * 4 * NB]
                        .rearrange("p (s x) -> p s x", s=4),
                        wbps[...].rearrange("p (s x) -> p s x",
                                            x=512)[:, :, 0:NB],
                        AF.Copy)

            def emit_builds(k):
                # A2 <- w*(-e) on DVE, then in-place +1 on ACT; B fully DVE
                wk = wsb[k % 2]
                Ak, Bk = A2[k % 2], B2[k % 2]
                tv = slice(k * TCH, (k + 1) * TCH)
                nbc = e3[:, tv, :].unsqueeze(2).broadcast_to(
                    [128, TCH, SLOTS, BL])
                abc = a3[:, tv, :].unsqueeze(2).broadcast_to(
                    [128, TCH, SLOTS, BL])
                wk3 = wk[...].rearrange("p (t n b) -> p t n b",
                                        n=SLOTS, b=BL)
                Ak3 = Ak[...].rearrange("p (t n b) -> p t n b",
                                        n=SLOTS, b=BL)
                Bk3 = Bk[...].rearrange("p (t n b) -> p t n b",
                                        n=SLOTS, b=BL)
                nc.vector.tensor_tensor(Ak3, wk3, nbc, OP.mult)
                nc.scalar.activation(Ak[...], Ak[...], AF.Copy,
                                     bias=1.0, scale=1.0)
                nc.vector.tensor_tensor(Bk3, wk3, abc, OP.mult)

            def emit_tree(k):
                # reads add-tree over slots on gpsimd (off critical path)
                p03 = p0t[...].rearrange("p (t n b) -> p t n b",
                                         n=SLOTS, b=BL)
                t1v = tr1[...].rearrange("p (t n b) -> p t n b", n=25, b=BL)
                t2v = tr2[...].rearrange("p (t n b) -> p t n b", n=12, b=BL)
                t3v = tr3[...].rearrange("p (t n b) -> p t n b", n=6, b=BL)
                t4v = tr4[...].rearrange("p (t n b) -> p t n b", n=3, b=BL)
                t5v = tr5[...].rearrange("p (t b) -> p t b", b=BL)
                t6v = tr6[...].rearrange("p (t b) -> p t b", b=BL)
                nc.gpsimd.tensor_add(t1v, p03[:, :, 0:25, :],
                                     p03[:, :, 25:50, :])
                nc.gpsimd.tensor_add(t2v, t1v[:, :, 0:12, :],
                                     t1v[:, :, 12:24, :])
                nc.gpsimd.tensor_add(t3v, t2v[:, :, 0:6, :],
                                     t2v[:, :, 6:12, :])
                nc.gpsimd.tensor_add(t4v, t3v[:, :, 0:3, :],
                                     t3v[:, :, 3:6, :])
                nc.gpsimd.tensor_add(t5v, t4v[:, :, 0, :], t4v[:, :, 1, :])
                nc.gpsimd.tensor_add(t6v, t5v, t4v[:, :, 2, :])
                nc.gpsimd.tensor_add(
                    reads_bs[:, k * TCH * BL:(k + 1) * TCH * BL]
                    .rearrange("p (t b) -> p t b", b=BL),
                    t6v, t1v[:, :, 24, :])

            emit_wchunk(0)
            emit_builds(0)
            for k in range(NCHK):
                Ak, Bk = A2[k % 2], B2[k % 2]
                wk = wsb[k % 2]
                for j in range(TCH):
                    ja, jb = j * NB, j * NB + HB
                    ma, mb = m2[2 * (j % 2)], m2[2 * (j % 2) + 1]
                    nc.vector.tensor_tensor(ma[...], st[:, ja:ja + HB],
                                            Ak[:, ja:ja + HB], OP.mult)
                    nc.vector.tensor_tensor(mb[...], st[:, jb:jb + HB],
                                            Ak[:, jb:jb + HB], OP.mult)
                    nc.vector.tensor_tensor(st[:, ja + NB:ja + NB + HB],
                                            ma[...], Bk[:, ja:ja + HB],
                                            OP.add)
                    nc.vector.tensor_tensor(st[:, jb + NB:jb + NB + HB],
                                            mb[...], Bk[:, jb:jb + HB],
                                            OP.add)
                if k + 1 < NCHK:
                    emit_wchunk(k + 1)
                    emit_builds(k + 1)
                # p0 = w_t * Mv_{t-1} on DVE (2x), tree on gpsimd
                nc.vector.tensor_tensor(p0t[...], st[:, 0:TCH * NB],
                                        wk[...], OP.mult)
                if k + 1 < NCHK:
                    nc.vector.tensor_copy(st[:, 0:NB],
                                          st[:, TCH * NB:(TCH + 1) * NB])
                emit_tree(k)

        # ---- P9: output head ----
        psB_stack = ExitStack()
        psB = psB_stack.enter_context(
            tc.tile_pool(name="psB", bufs=1, space="PSUM"))
        for c in range(4):
            sl = slice(c * 400, (c + 1) * 400)
            fp = psB.tile([DK, 400], F32, tag="mm2", bufs=4)
            nc.tensor.matmul(fp[...], fw1t[...], reads_bs[:, sl],
                             start=True, stop=False)
            nc.tensor.matmul(fp[...], fw2t[...], kbar[:, sl],
                             start=False, stop=True)
            nc.scalar.activation(f_all[:, sl], fp[...], AF.Tanh,
                                 bias=fb[...], scale=1.0)
        for c in range(4):
            sl = slice(c * 400, (c + 1) * 400)
            pp = psB.tile([1, 400], F32, tag="mm1", bufs=2)
            nc.tensor.matmul(pp[...], pwt[...], f_all[:, sl])
            nc.scalar.activation(out_sb[:, sl], pp[...], AF.Sigmoid,
                                 bias=pb[...], scale=1.0)
        nc.sync.dma_start(out_d[...], out_sb[...])
        psB_stack.close()

    nc.finalize()
    return nc


def _host_inputs(inputs):
    """Build per-core + replicated DRAM inputs from the full problem inputs."""
    bf = np.float16
    qs = np.asarray(inputs["question_seq"]).astype(np.int64)
    cs = np.asarray(inputs["correctness_seq"]).astype(np.int64)
    q2c = np.asarray(inputs["q2c_table"]).astype(np.int32)
    q2m = np.asarray(inputs["q2c_mask"]).astype(np.int32)
    ke = np.asarray(inputs["key_embed"], np.float32)
    ve = np.asarray(inputs["value_embed"], np.float32)
    mk = np.asarray(inputs["Mk"], np.float32)
    mv0 = np.asarray(inputs["Mv0"], np.float32)
    fw = np.asarray(inputs["f_W"], np.float32)
    fb = np.asarray(inputs["f_b"], np.float32)
    ew = np.asarray(inputs["e_W"], np.float32)
    eb = np.asarray(inputs["e_b"], np.float32)
    aw = np.asarray(inputs["a_W"], np.float32)
    ab = np.asarray(inputs["a_b"], np.float32)
    pw = np.asarray(inputs["p_W"], np.float32)
    pb = np.asarray(inputs["p_b"], np.float32)

    # [CP, C*DK] chunked-contiguous table layouts (chunk c rows 125c..125c+124)
    kep = np.zeros((128, KCH, DK), np.float16)
    kep[0:CP] = ke.astype(np.float16).reshape(KCH, CP, DK).transpose(1, 0, 2)
    kep = kep.reshape(128, KCH * DK)
    vep = np.zeros((128, VCH, DK), np.float16)
    vep[0:CP] = ve.astype(np.float16).reshape(VCH, CP, DK).transpose(1, 0, 2)
    vep = vep.reshape(128, VCH * DK)

    rep = {
        "q2c_comb": np.stack([q2c.T, q2m.T], 2).reshape(4, 2 * NUM_Q)
        .astype(np.int16),
        "ket": np.ascontiguousarray(kep),
        "vet": np.ascontiguousarray(vep),
        "iof": np.arange(128, dtype=np.float32).reshape(128, 1),
        "mkt": mk.T.astype(bf),
        "ewt": ew.T.astype(bf),
        "awt": aw.T.astype(bf),
        "fw1t": fw[:, :DK].T.astype(bf),
        "fw2t": fw[:, DK:].T.astype(bf),
        "pwt": pw.T.astype(bf),
        "eb": eb.reshape(DK, 1).astype(np.float32),
        "ab": ab.reshape(DK, 1).astype(np.float32),
        "fb": fb.reshape(DK, 1).astype(np.float32),
        "pb": pb.reshape(1, 1).astype(np.float32),
        "mv0r": np.repeat(mv0.T, BL, axis=1).astype(bf),
    }
    in_maps = []
    for core in range(NCORES):
        q_flat = qs[core * BL:(core + 1) * BL].T.reshape(-1)   # t-major
        c_flat = cs[core * BL:(core + 1) * BL].T.reshape(-1)
        # per-gpsimd-core index lists: core k takes bs [200k, 200k+200),
        # padded to NIX and wrapped into its 16 partitions
        qwa = np.zeros((8, NIX), np.int16)
        qwa[:, 0:200] = q_flat.reshape(8, 200)
        qwa = qwa.reshape(8, NIX // 16, 16).transpose(0, 2, 1) \
            .reshape(128, NIX // 16)
        m = dict(rep)
        m["qseq_w"] = np.ascontiguousarray(qwa)
        m["corrf"] = np.broadcast_to(c_flat.astype(np.float32),
                                     (4, BS)).copy()
        in_maps.append(m)
    return in_maps


def kernel(**inputs):
    global _PROG
    if _PROG is None:
        _PROG = _build_program()
    in_maps = _host_inputs(inputs)
    res = run_bass_kernel_spmd(_PROG, in_maps, core_ids=list(range(NCORES)))
    out = np.zeros((B, S), np.float32)
    for core in range(NCORES):
        o = res.results[core]["out"].reshape(S, BL)
        out[core * BL:(core + 1) * BL] = o.T
    return out
